# revision 111
# baseline (speedup 1.0000x reference)
"""Trainium2 Bass kernel for MiniCPMV ViT window-attention + 2x2 merger block.

Architecture (per reference):
  x[1,16384,1152] -> LN1 -> 2x2-window reorder -> QKV -> 4-token window attn
  (16 heads x 72) -> out-proj -> un-reorder + residual -> re-reorder ->
  [4096 windows x 4608] -> LN2 -> Linear(4608->17216) -> gelu(tanh) ->
  Linear(17216->1152) -> + mean-pool residual -> [1,4096,1152]

Key observation: the un-reorder after attention and the re-reorder before the
merger cancel, so everything stays in window order end-to-end and the output
is already in window (= merged token) order.

Sharding: pure data parallel over 8 cores; each core takes 2048 tokens
(512 windows, half of one image = 16 window-rows), weights replicated.
Token order within a core is (a, w): a = position-in-window (0..3),
w = window index (0..511), so per-a slices are contiguous.

On-chip layout is feature-major ([d on partitions, tokens on free axis]);
host pre-transposes x (bf16, chunk-major partition-leading so each chunk is
two fused DMAs) and all weights. QKV and the out-projection run in fp8-e4m3
DoubleRow mode (2 k-tiles per matmul; weights pre-scaled x32 host-side, the
scale folded out at PSUM evacuation); W1/W2 stay bf16 for the 2e-2 accuracy
budget. LN stat chains and h2 writes ride the otherwise-idle Pool (gpsimd)
engine; each chunk's LN2 is deferred past the next chunk's scores products
so the DVE queue serves the attention chain first. Stage-B (merger MLP)
weights stream on the Activation HWDGE queue, prefetched one item ahead, and
block 0 is interleaved into late stage-A chunks; the last beta block and the
gamma half-block alternate their W2/output chains so PE covers the final acc
updates. Output leaves the device feature-major and is transposed on host.
"""

import numpy as np
import ml_dtypes

import concourse.bacc as bacc
import concourse.tile as tile
import concourse.bass as bass
from concourse import mybir

F32 = mybir.dt.float32
BF16 = mybir.dt.bfloat16
F8 = mybir.dt.float8e4
I32 = mybir.dt.int32
AF = mybir.ActivationFunctionType
ALU = mybir.AluOpType
DR = mybir.MatmulPerfMode.DoubleRow

# Problem constants (hardcoded per spec)
B, H, W, D, I, NH = 4, 64, 64, 1152, 4304, 16
T = B * H * W          # 16384 tokens
HD = D // NH           # 72 head dim
NCORES = 8
TS = T // NCORES       # 2048 tokens per core
NW = TS // 4           # 512 windows per core
DT = D // 128          # 9 feature tiles
JQ = 3 * D // 128      # 27 qkv output tiles
D4 = 4 * D             # 4608 merged feature dim
KT1 = D4 // 128        # 36 contraction tiles for w1
J1 = 4 * I             # 17216
J1P = 17280            # padded to 135*128
JT1 = J1P // 128       # 135
JBLK = 15              # w1 j-tiles per block
NBLK = JT1 // JBLK     # 9 blocks
CH = 8                 # stage-A chunks
WC = NW // CH          # 64 windows per chunk
TC = 4 * WC            # 256 tokens per chunk
EPS = 1e-6
SM_SCALE = 1.0 / np.sqrt(HD)
S_WQ = 32.0               # fp8 qkv weight pre-scale (folded out at PSUM evac)
S_WO = 32.0               # fp8 out-proj weight pre-scale

PHASE_MARKS = []  # (label, first_instruction_index); for profiling only


def _mark(nc, label):
    PHASE_MARKS.append((label, int(nc.get_next_instruction_name()[2:])))


def _rsqrt(nc, st_pool, out, z, tag, iters=3, eng=None, sdt=None):
    """out = 1/sqrt(z) via Newton from y0=1 (valid for z in ~(0.1, 3)).

    LN variances here are ~1 so a constant init converges in 3 iterations.
    Four scratch tags scheduled so no tag is reused while its tile is still
    an input of a later instruction (safe with bufs=1 pools).
    z: [1, N] f32 SBUF; out: [1, N] bf16.
    """
    ve = eng if eng is not None else nc.vector
    dt_ = sdt if sdt is not None else F32
    n = z.shape[-1]
    seq = [0, 1, 2, 3, 1, 2, 3, 0, 1, 2, 3]  # tag index per scratch alloc
    ti = 0

    def scratch():
        nonlocal ti
        t = st_pool.tile([1, n], dt_, tag=f"{tag}_{seq[ti]}", name=f"{tag}_s{ti}", bufs=1)
        ti += 1
        return t

    with nc.allow_low_precision(reason="rsqrt scratch"):
        # y1 = 1.5 - 0.5 z   (Newton step from y0 = 1)
        y = scratch()
        ve.tensor_scalar(y, z, -0.5, 1.5, ALU.mult, ALU.add)
        for it in range(1, iters):
            t = scratch()
            ve.tensor_mul(t, z, y)
            t2 = scratch()
            ve.tensor_mul(t2, t, y)
            u = scratch()
            ve.tensor_scalar(u, t2, -0.5, 1.5, ALU.mult, ALU.add)
            if it < iters - 1:
                yn = scratch()
                ve.tensor_mul(yn, u, y)
            else:
                yn = out
                ve.tensor_mul(yn, u, y)
            y = yn


def build_program(debug=False):
    """Build the single-core SPMD program (same NEFF on all 8 cores)."""
    from contextlib import ExitStack
    PHASE_MARKS.clear()

    nc = bacc.Bacc("TRN2", target_bir_lowering=False, num_devices=NCORES)

    # ---- DRAM parameters -------------------------------------------------
    def inp(name, shape, dtype):
        return nc.dram_tensor(name, shape, dtype, kind="ExternalInput").ap()

    xT_d = inp("xT", [CH, 128, DT, 4 * WC], BF16)   # chunk-major, partition-leading
    wqkv_d = inp("wqkv", [128, JQ, DT, 128], F8)    # [p][jt][dt][col], x S_WQ
    bqkv_d = inp("bqkv", [128, JQ], F32)
    wo_d = inp("wo", [128, DT, D], F8)              # [p][dtk][j], x S_WO
    bo_d = inp("bo", [128, DT], F32)
    obd_d = inp("ones_bd", [128, DT, NH], BF16)     # block-diag head masks
    obdT_d = inp("ones_bdT", [NH, DT, 128], BF16)
    w1_d = inp("w1t", [JT1, 128, KT1, 128], BF16)   # [jt][p][kt][col]
    b1_d = inp("b1", [128, JT1], F32)
    w2_d = inp("w2t", [DT, 128, JT1, 128], BF16)    # [dt][p][jt][col]
    b2_d = inp("b2", [128, DT], F32)

    # feature-major output [dt][p][w]; host transposes to [NW, D] after gather
    out_d = nc.dram_tensor("out", [DT, 128, NW], F32, kind="ExternalOutput").ap()
    if debug:
        dbg = {
            "dbg_qk": nc.dram_tensor("dbg_qk", [128, 2 * DT, TC], BF16,
                                     kind="ExternalOutput").ap(),
            "dbg_h": nc.dram_tensor("dbg_h", [128, DT, TC], F8,
                                    kind="ExternalOutput").ap(),
            "dbg_xc": nc.dram_tensor("dbg_xc", [128, DT, 4, WC], BF16,
                                     kind="ExternalOutput").ap(),
            "dbg_v": nc.dram_tensor("dbg_v", [128, DT, TC], BF16,
                                    kind="ExternalOutput").ap(),
            "dbg_attn": nc.dram_tensor("dbg_attn", [NH, 4, TS // 4 // CH, 4], BF16,
                                       kind="ExternalOutput").ap(),
            "dbg_y": nc.dram_tensor("dbg_y", [128, DT, 4, TS // 4 // CH], BF16,
                                    kind="ExternalOutput").ap(),
            "dbg_h2": nc.dram_tensor("dbg_h2", [128, DT, 4, NW], BF16,
                                     kind="ExternalOutput").ap(),
            "dbg_res": nc.dram_tensor("dbg_res", [128, DT, NW], BF16,
                                      kind="ExternalOutput").ap(),
            "dbg_acc": nc.dram_tensor("dbg_acc", [128, DT, NW], F32,
                                      kind="ExternalOutput").ap(),
        }

    with tile.TileContext(nc) as tc, ExitStack() as ctx:
        # ---- pools -------------------------------------------------------
        consts = ctx.enter_context(tc.tile_pool(name="consts", bufs=1))
        persist = ctx.enter_context(tc.tile_pool(name="persist", bufs=1))

        # ---- constants (DMAs deferred until after chunk 0's x-load) ------
        wo_sb = consts.tile([128, DT, D], F8)
        bqkv_sb = consts.tile([128, JQ], F32)
        bo_sb = consts.tile([128, DT], F32)
        b1_sb = consts.tile([128, JT1], F32)
        b2_sb = consts.tile([128, DT], F32)
        obd_sb = consts.tile([128, DT, NH], BF16)
        obdT_sb = consts.tile([NH, DT, 128], BF16)

        def load_consts():
            nc.sync.dma_start(obd_sb, obd_d)
            nc.sync.dma_start(bqkv_sb, bqkv_d)

        def load_consts2():
            nc.sync.dma_start(obdT_sb, obdT_d)
            nc.sync.dma_start(wo_sb, wo_d)
            nc.sync.dma_start(bo_sb, bo_d)

        def load_consts3():
            nc.sync.dma_start(b1_sb, b1_d)
            nc.sync.dma_start(b2_sb, b2_d)

        ones_col = consts.tile([128, 1], BF16)
        nc.vector.memset(ones_col, 1.0)

        # attention -> merger handoff, kept in SBUF
        h2 = persist.tile([128, DT, 4, NW], BF16)    # LN2-normalized y

        # ---- stage-B shared pools (used interleaved with A and after) ----
        acc_pool = ctx.enter_context(tc.tile_pool(name="acc", bufs=1))
        w1_pool = ctx.enter_context(tc.tile_pool(name="w1s", bufs=2))
        m2h_pool = ctx.enter_context(tc.tile_pool(name="m2h", bufs=1))
        w2_pool = ctx.enter_context(tc.tile_pool(name="w2s", bufs=3))
        ps_b = ctx.enter_context(tc.tile_pool(name="ps_b", bufs=2, space="PSUM"))
        m2f_pool = [None]

        acc = acc_pool.tile([128, DT, NW], F32)
        HW1 = NW // 2          # window-half size (256)
        NIB = 1                # blocks interleaved into stage A at N=HW1

        def w1_compute(jt, wlo, n, dst, w1s, sfx):
            """One W1 j-tile over window range [wlo, wlo+n) -> dst [128, n]."""
            mm = ps_b.tile([128, n], F32, tag="bmm", name=f"bmm{sfx}")
            for kt in range(KT1):
                a, dt = divmod(kt, DT)
                nc.tensor.matmul(mm, w1s[:, kt], h2[:, dt, a, wlo:wlo + n],
                                 start=(kt == 0), stop=(kt == KT1 - 1))
            nc.scalar.activation(dst, mm, AF.Gelu_apprx_tanh,
                                 bias=b1_sb[:, jt:jt + 1])

        def acc_update(dt, mm_ap, wlo, n, first, last):
            # acc is pre-seeded with the mean-pool residual at h2c time, so
            # every block update is an add (the last one also adds the bias)
            accs = acc[:, dt, wlo:wlo + n]
            if last:
                nc.vector.scalar_tensor_tensor(
                    accs, mm_ap, b2_sb[:, dt:dt + 1], accs, ALU.add, ALU.add)
            else:
                nc.vector.tensor_add(accs, mm_ap, accs)

        def fin_dt(dt, mts):
            # output is feature-major: DMA the finished window-half directly
            lo, hi = mts[0] * 128, (mts[-1] + 1) * 128
            nc.sync.dma_start(out_d[dt, :, lo:hi], acc[:, dt, lo:hi])

        def w2_compute(blk, dt, wlo, n, m2t, flags, w2s, fin_mts, sfx):
            """W2 for one (block, dt) over [wlo, wlo+n); flags: [(lo,sz,first,last)].

            fin_mts: if set, emit that dt's output DMA right after its
            last acc update (folds the output tail into the last block).
            """
            mm = ps_b.tile([128, n], F32, tag="bmm", name=f"w2mm{sfx}_{dt}")
            for j in range(JBLK):
                nc.tensor.matmul(mm, w2s[:, j], m2t[:, j],
                                 start=(j == 0), stop=(j == JBLK - 1))
            for lo, sz, first, last in flags:
                acc_update(dt, mm[:, lo - wlo:lo - wlo + sz], lo, sz,
                           first, last)
            if fin_mts is not None:
                fin_dt(dt, fin_mts)

        # ---- unified stage-B worklist: interleave / beta / gamma ----------
        # item: ("w1"/"w1g", blk, j, lo, n) or ("w2", blk, dt, lo, n, flags, fin)
        items = []
        for blk in range(NIB):          # interleave: block 0, window half 1
            for j in range(JBLK):
                items.append(("w1", blk, j, 0, HW1))
            for dt in range(DT):
                items.append(("w2", blk, dt, 0, HW1,
                              [(0, HW1, blk == 0, False)], None))
        N_ILV = len(items)              # only these may be emitted inside A
        for blk in range(NIB, NBLK - 1):  # beta: full-N blocks
            for j in range(JBLK):
                # first beta items compute window half 1 first so they don't
                # stall on the final h2 chunk right at the A->B boundary
                kind = "w1x" if blk == NIB and j < 2 else "w1"
                items.append((kind, blk, j, 0, NW))
            for dt in range(DT):
                items.append(("w2", blk, dt, 0, NW,
                              [(0, HW1, False, False),
                               (HW1, HW1, blk == NIB, False)],
                              None))
        # tail: last beta block (B{NBLK-1}) + gamma (block 0 half 2) with
        # their w2/fin chains interleaved so PE covers each other's acc DVE
        blk = NBLK - 1
        for j in range(JBLK):
            items.append(("w1", blk, j, 0, NW))
        for j in range(JBLK):
            items.append(("w1g", 0, j, HW1, HW1))
        for dt in range(DT):
            items.append(("w2", blk, dt, 0, NW,
                          [(0, HW1, False, True), (HW1, HW1, False, False)],
                          [0, 1]))
            items.append(("w2", 0, dt, HW1, HW1,
                          [(HW1, HW1, False, True)],
                          [2, 3]))

        m2_of = {}
        wpre = {}
        bbudget = [0]
        pi = [0]

        def load_w(i):
            """Issue the weight DMA for item i (idempotent prefetch)."""
            if i >= len(items) or i in wpre:
                return
            it = items[i]
            # stage-B weights ride the Activation HWDGE queue so they never
            # delay the latency-sensitive x/wq stream on the SP queue
            if it[0] != "w2":
                jt = it[1] * JBLK + it[2]
                t = w1_pool.tile([128, KT1, 128], BF16, tag="w1s",
                                 name=f"w1s_{i}")
                nc.scalar.dma_start(t, w1_d[jt])
            else:
                blk, dt = it[1], it[2]
                t = w2_pool.tile([128, JBLK, 128], BF16, tag="w2s",
                                 name=f"w2s_{i}")
                nc.scalar.dma_start(t, w2_d[dt, :, blk * JBLK:(blk + 1) * JBLK])
            wpre[i] = t

        def emit_bitem():
            if pi[0] >= len(items) or bbudget[0] <= 0:
                return
            bbudget[0] -= 1
            i = pi[0]
            pi[0] += 1
            it = items[i]
            load_w(i)
            load_w(i + 1)
            if i + 2 < len(items) and items[i + 2][0] == "w2" and \
                    items[i + 1][0] == "w2":
                load_w(i + 2)   # w2 pool has 3 bufs: run 2 ahead
            if it[0] == "w2" and it[2] in (0, 3, 6):
                # w1 bufs are both free during a block's w2 run: top up the
                # next block's first w1 weights early. Cap outstanding w1
                # prefetches at 2 (pool bufs) or the blocked DMA would sit at
                # the queue head and stall the w2 weight stream behind it.
                n_w1_pre = sum(1 for k2 in wpre if items[k2][0] != "w2")
                if n_w1_pre < 2:
                    for k in range(i + 2, min(i + 16, len(items))):
                        if items[k][0] != "w2" and k not in wpre:
                            load_w(k)
                            break
            w = wpre.pop(i)
            if it[0] != "w2":
                _, blk, j, lo, n = it
                if j == 0:
                    if n == HW1:
                        m2_of[blk] = m2h_pool.tile([128, JBLK, HW1], BF16,
                                                   tag="m2h", name=f"m2h{i}")
                    else:
                        m2_of[blk] = m2f_pool[0].tile([128, JBLK, NW], BF16,
                                                      tag="m2f", name=f"m2f{i}")
                if it[0] == "w1x":
                    for half in range(2):
                        w1_compute(blk * JBLK + j, half * HW1, HW1,
                                   m2_of[blk][:, j, half * HW1:(half + 1) * HW1],
                                   w, f"b{i}_{half}")
                else:
                    w1_compute(blk * JBLK + j, lo, n, m2_of[blk][:, j], w,
                               f"b{i}")
            else:
                _, blk, dt, lo, n, flags, fin = it
                w2_compute(blk, dt, lo, n, m2_of[blk], flags, w, fin, f"b{i}")
                if dt == DT - 1:
                    m2_of.pop(blk)

        # =================== Stage A: LN1 + attention =====================
        with ExitStack() as actx:
            xc_pool = actx.enter_context(tc.tile_pool(name="xc", bufs=3))
            wq_pool = actx.enter_context(tc.tile_pool(name="wq", bufs=4))
            h_pool = actx.enter_context(tc.tile_pool(name="h", bufs=2))
            ob_pool = actx.enter_context(tc.tile_pool(name="ob", bufs=1))
            qk_pool = actx.enter_context(tc.tile_pool(name="qk", bufs=1))
            v_pool = actx.enter_context(tc.tile_pool(name="v", bufs=2))
            p_pool = actx.enter_context(tc.tile_pool(name="p", bufs=2))
            sm_pool = actx.enter_context(tc.tile_pool(name="sm", bufs=2))
            av_pool = actx.enter_context(tc.tile_pool(name="av", bufs=2))
            st_pool = actx.enter_context(tc.tile_pool(name="st", bufs=2))
            st1_pool = actx.enter_context(tc.tile_pool(name="st1b", bufs=1))
            ps_mm = actx.enter_context(tc.tile_pool(name="ps_mm", bufs=3, space="PSUM"))
            ps_work = actx.enter_context(tc.tile_pool(name="ps_work", bufs=3, space="PSUM"))

            def ln_stage(c):
                """Emit x-load + LN1 + normalize for chunk c; returns (xc, h)."""
                w0 = c * WC
                _mark(nc, f"A{c}:xload")
                # -- load x chunk: [128, dt, a, WC] bf16, two fused DMAs
                xc = xc_pool.tile([128, DT, 4, WC], BF16, tag="xc", name=f"xc{c}")
                xf = xc.rearrange("p d a w -> p d (a w)")
                xs = xT_d[c]
                if c == 0:
                    # finer granularity at cold start: stats can begin as soon
                    # as the first dt tiles land
                    for lo, hi in ((0, 3), (3, 6), (6, 9)):
                        nc.sync.dma_start(xf[:, lo:hi], xs[:, lo:hi])
                else:
                    nc.sync.dma_start(xf[:, :5], xs[:, :5])
                    nc.sync.dma_start(xf[:, 5:], xs[:, 5:])
                if debug and c == 0:
                    nc.sync.dma_start(dbg["dbg_xc"], xc)

                _mark(nc, f"A{c}:ln1")
                # -- LN1 stats: col sums of x and x^2 via ones-vector matmul
                st1 = ps_work.tile([1, 2 * TC], F32, tag="work", name=f"st1_{c}")
                for dt in range(DT):
                    xflat = xc[:, dt].rearrange("p a w -> p (a w)")
                    xsq = st_pool.tile([128, TC], BF16, tag="xsq", name=f"xsq{c}_{dt}", bufs=1)
                    nc.vector.tensor_mul(xsq, xflat, xflat)
                    # NOTE: start=True clears has_written for the WHOLE psum
                    # bank, so only the bank's first matmul may set it; other
                    # groups overwrite-on-first-touch via the cleared bits.
                    nc.tensor.matmul(st1[:, :TC], ones_col, xflat,
                                     start=(dt == 0), stop=(dt == DT - 1))
                    nc.tensor.matmul(st1[:, TC:], ones_col, xsq,
                                     start=False, stop=(dt == DT - 1))
                stx, stq = st1[:, :TC], st1[:, TC:]

                # var = stq/D - (stx/D)^2 ; rstd = 1/sqrt(var) via DVE Newton
                # (DVE ops may read at most one PSUM input -> evac via ts_mul)
                m1t = st1_pool.tile([1, TC], F32, tag="m1t", name=f"m1t_{c}")
                nc.vector.tensor_scalar_mul(m1t, stx, 1.0 / D)
                var = st1_pool.tile([1, TC], F32, tag="var", name=f"var_{c}")
                nc.vector.tensor_scalar_mul(var, stq, 1.0 / D)
                # (Pool has no scalar_tensor_tensor: use mul/sub pairs and a
                # positive pmu = mean*rstd, subtracted in the normalize)
                t1 = st1_pool.tile([1, TC], F32, tag="r1_0", name=f"t1_{c}")
                nc.gpsimd.tensor_mul(t1, m1t, m1t)
                nc.gpsimd.tensor_sub(var, var, t1)
                rstd = st_pool.tile([1, TC], BF16, tag="rstd", name=f"rstd_{c}")
                _rsqrt(nc, st1_pool, rstd, var, "r1", iters=2, eng=nc.gpsimd,
                       sdt=BF16)
                pmu = st_pool.tile([1, TC], BF16, tag="nmu", name=f"nmu_{c}")
                with nc.allow_low_precision(reason="ln scale bf16"):
                    nc.gpsimd.tensor_mul(pmu, m1t, rstd)

                rstd_b = st_pool.tile([128, TC], BF16, tag="rstd_b", name=f"rstdb_{c}")
                pmu_b = st_pool.tile([128, TC], BF16, tag="nmu_b", name=f"nmub_{c}")
                nc.gpsimd.partition_broadcast(rstd_b, rstd)
                nc.gpsimd.partition_broadcast(pmu_b, pmu)

                _mark(nc, f"A{c}:norm")
                # -- normalize -> h fp8 [128, dt, TC] (qkv runs in fp8 DoubleRow)
                h = h_pool.tile([128, DT, TC], F8, tag="h", name=f"h{c}")
                for dt in range(DT):
                    tmp = st_pool.tile([128, TC], BF16, tag="normtmp", name=f"nt{c}_{dt}", bufs=1)
                    nc.vector.tensor_mul(tmp, xc[:, dt].rearrange("p a w -> p (a w)"),
                                         rstd_b)
                    with nc.allow_low_precision(reason="qkv input fp8"):
                        nc.vector.tensor_sub(h[:, dt], tmp, pmu_b)
                return xc, h

            qkv_tiles = {}
            wq_pend = {}

            def qkv_stage(c, h, jplo=0, jphi=JQ):
                """QKV matmuls for chunk c (pair range [jplo, jphi))."""
                _mark(nc, f"A{c}:qkv")
                if c not in qkv_tiles:
                    qk = qk_pool.tile([128, 2 * DT, TC], BF16, tag="qk",
                                      name=f"qk{c}")
                    vt = v_pool.tile([128, DT, TC], BF16, tag="v", name=f"v{c}")
                    qkv_tiles[c] = (qk, vt)
                    wq_pend[c] = {}
                qk, vt = qkv_tiles[c]
                def wq_load(jg):
                    """One fused DMA for a group of 4 consecutive j-tiles."""
                    ng = min(4, JQ - jg)
                    wqg = wq_pool.tile([128, 4, DT, 128], F8, tag="wq",
                                       name=f"wqg{c}_{jg}")
                    nc.sync.dma_start(
                        wqg[:, :ng].rearrange("p j d k -> p j (d k)"),
                        wqkv_d[:, jg:jg + ng].rearrange("p j d k -> p j (d k)"))
                    wq_pend[c][jg] = wqg

                for jp in range(jplo, jphi, 2):
                    nj = min(2, JQ - jp)
                    jg = (jp // 4) * 4
                    if jg not in wq_pend[c]:
                        wq_load(jg)
                    if jp == jg and jg + 4 < JQ and jg + 4 not in wq_pend[c]:
                        wq_load(jg + 4)
                    wqg = wq_pend[c][jg]
                    wqs = [wqg[:, jp - jg + j] for j in range(nj)]
                    mm = ps_mm.tile([128, 2, TC], F32, tag="mm0",
                                    name=f"qmm{c}_{jp}")
                    for dtp in range(0, DT, 2):
                        dr = dtp + 2 <= DT
                        for j in range(nj):
                            if dr:
                                nc.tensor.matmul(mm[:, j], wqs[j][:, dtp:dtp + 2],
                                                 h[:, dtp:dtp + 2],
                                                 start=(dtp == 0 and j == 0),
                                                 stop=(dtp + 2 >= DT),
                                                 perf_mode=DR)
                            else:
                                nc.tensor.matmul(mm[:, j], wqs[j][:, dtp],
                                                 h[:, dtp],
                                                 start=(dtp == 0 and j == 0),
                                                 stop=True)
                    for j in range(nj):
                        jt = jp + j
                        dstt = qk[:, jt] if jt < 2 * DT else vt[:, jt - 2 * DT]
                        nc.scalar.activation(dstt, mm[:, j], AF.Identity,
                                             bias=bqkv_sb[:, jt:jt + 1],
                                             scale=1.0 / S_WQ)
                if debug and c == 0 and jphi >= JQ:
                    nc.sync.dma_start(dbg["dbg_qk"], qk)
                    nc.sync.dma_start(dbg["dbg_v"], vt)
                    nc.sync.dma_start(dbg["dbg_h"], h)
                return qk, vt

            # 2-deep software pipeline: LN two chunks ahead, QKV one ahead
            lns = {0: ln_stage(0)}
            load_consts()
            lns[1] = ln_stage(1)
            qkvs = {0: qkv_stage(0, lns[0][1])}
            load_consts2()
            pend_ln2 = [None]
            for c in range(CH):
                w0 = c * WC
                xc, h = lns.pop(c)
                qk, vt = qkvs.pop(c)
                ilv = c >= 4    # h2 half 1 complete -> can interleave B work
                if ilv:
                    rem = max(0, N_ILV - pi[0] - 1)
                    bbudget[0] = -(-rem // (CH - c))  # spread evenly

                _mark(nc, f"A{c}:scores")
                # -- scores: p = q (x) k -> block-diag head reduce -> scs psum
                scs = [ps_work.tile([16, 2, 4, WC], F32, tag="work",
                                    name=f"scs{c}_{i}") for i in range(2)]
                for dt in range(DT):
                    q3 = qk[:, dt].rearrange("p (a w) -> p a w", a=4)
                    k3 = qk[:, DT + dt].rearrange("p (a w) -> p a w", a=4)
                    p_t = p_pool.tile([128, 4, 4, WC], BF16)
                    nc.vector.tensor_mul(
                        p_t,
                        q3.unsqueeze(2).to_broadcast([128, 4, 4, WC]),
                        k3.unsqueeze(1).to_broadcast([128, 4, 4, WC]),
                    )
                    for qi in range(4):
                        nc.tensor.matmul(scs[qi // 2][:, qi % 2], obd_sb[:, dt],
                                         p_t[:, qi].rearrange("p a w -> p (a w)"),
                                         start=(dt == 0 and qi % 2 == 0),
                                         stop=(dt == DT - 1))
                # a few qkv pairs ahead of the deferred LN2 so PE is not
                # blocked in-order on the ln2 stats' ysq dependency
                if c + 1 < CH:
                    qkvs[c + 1] = qkv_stage(c + 1, lns[c + 1][1], 0, 8)
                if pend_ln2[0] is not None:
                    pend_ln2[0]()       # previous chunk's deferred LN2+h2c
                    pend_ln2[0] = None
                if ilv:
                    emit_bitem()

                _mark(nc, f"A{c}:softmax")
                # -- softmax over ki: exp (fused scale), sum, reciprocal, scale
                # layout [h,qi,ki,w]: transpose-free exp evac + packed-w 2x mul
                esb = sm_pool.tile([16, 4, 4, WC], BF16, tag="esb")
                for half in range(2):
                    nc.scalar.activation(
                        esb[:, 2 * half:2 * half + 2], scs[half],
                        AF.Exp, scale=float(SM_SCALE),
                    )
                den = sm_pool.tile([16, 4, WC], BF16, tag="den", bufs=1)
                with nc.allow_low_precision(reason="4-elem softmax sum"):
                    nc.vector.tensor_reduce(den, esb.transpose([0, 1, 3, 2]),
                                            axis=mybir.AxisListType.X,
                                            op=ALU.add)
                rden = sm_pool.tile([16, 4, WC], BF16, tag="rden", bufs=1)
                with nc.allow_low_precision(reason="softmax recip bf16"):
                    nc.vector.reciprocal(rden, den)
                attn = sm_pool.tile([16, 4, 4, WC], BF16, tag="attn")
                nc.vector.tensor_mul(
                    attn, esb,
                    rden.unsqueeze(2).to_broadcast([16, 4, 4, WC]),
                )
                if debug and c == 0:
                    nc.sync.dma_start(dbg["dbg_attn"], attn)
                if c + 1 < CH:
                    qkvs[c + 1] = qkv_stage(c + 1, lns[c + 1][1], 8, 16)
                if ilv:
                    emit_bitem()

                _mark(nc, f"A{c}:av")
                # -- AV: expand attn to feature rows (PE), evac, mul v, 2-add
                o_bf = ob_pool.tile([128, DT, 4, WC], F8, tag="ob", name=f"ob{c}")
                for dt in range(DT):
                    v3 = vt[:, dt].rearrange("p (a w) -> p a w", a=4)
                    exb = av_pool.tile([128, 4, 4, WC], BF16, tag="exb")
                    for half in range(2):
                        ex = ps_work.tile([128, 2, 4, WC], F32, tag="work",
                                          name=f"ex{c}_{dt}_{half}")
                        for qj in range(2):
                            nc.tensor.matmul(
                                ex[:, qj], obdT_sb[:, dt],
                                attn[:, 2 * half + qj],
                                start=True, stop=True,
                            )
                        nc.scalar.activation(exb[:, 2 * half:2 * half + 2], ex,
                                             AF.Identity)
                    prod = av_pool.tile([128, 4, 4, WC], BF16, tag="prod", bufs=1)
                    nc.vector.tensor_mul(
                        prod, exb,
                        v3.unsqueeze(1).to_broadcast([128, 4, 4, WC]),
                    )
                    t2 = av_pool.tile([128, 4, 2, WC], BF16, tag="t2", bufs=1)
                    nc.vector.tensor_add(t2, prod[:, :, 0:2], prod[:, :, 2:4])
                    with nc.allow_low_precision(reason="o fp8 for fp8 outproj"):
                        nc.vector.tensor_add(o_bf[:, dt], t2[:, :, 0],
                                             t2[:, :, 1])
                    if ilv and dt in (3, 7):
                        emit_bitem()

                # independent PE work before outproj (which waits on av DVE)
                if c + 1 < CH:
                    qkvs[c + 1] = qkv_stage(c + 1, lns[c + 1][1], 16, JQ)
                if c + 2 < CH:
                    lns[c + 2] = ln_stage(c + 2)
                if ilv:
                    emit_bitem()

                _mark(nc, f"A{c}:outproj")
                # -- out-projection + bias + residual -> y (in-place into xc)
                # dtk-outer within dto-pairs: PE starts on o_bf[0] early.
                y = xc.rearrange("p d a w -> p d (a w)")
                o_f = o_bf.rearrange("p d a w -> p d (a w)")
                for g in range(0, DT, 2):
                    nd = min(2, DT - g)
                    opt = ps_work.tile([128, 2, TC], F32, tag="work",
                                       name=f"op{c}_{g}")
                    mms = [opt[:, i] for i in range(nd)]
                    for dtp in range(0, DT, 2):
                        dr = dtp + 2 <= DT
                        for i in range(nd):
                            cols = slice((g + i) * 128, (g + i + 1) * 128)
                            if dr:
                                nc.tensor.matmul(
                                    mms[i], wo_sb[:, dtp:dtp + 2, cols],
                                    o_f[:, dtp:dtp + 2],
                                    start=(dtp == 0 and i == 0),
                                    stop=(dtp + 2 >= DT), perf_mode=DR)
                            else:
                                nc.tensor.matmul(
                                    mms[i], wo_sb[:, dtp, cols], o_f[:, dtp],
                                    start=(dtp == 0 and i == 0), stop=True)
                    for i in range(nd):
                        with nc.allow_low_precision(reason="residual stream bf16"):
                            ot = st_pool.tile([128, TC], BF16, tag="opt",
                                              bufs=1, name=f"ot{c}_{g}_{i}")
                            nc.vector.tensor_scalar(
                                ot, mms[i], 1.0 / S_WO,
                                bo_sb[:, g + i:g + i + 1], ALU.mult, ALU.add)
                            nc.vector.tensor_add(y[:, g + i], y[:, g + i], ot)
                if debug and c == 0:
                    nc.sync.dma_start(dbg["dbg_y"], xc)
                if ilv:
                    emit_bitem()

                def make_ln2(c, xc, y, w0, ilv):
                    def emit_ln2():
                        _mark(nc, f"A{c}:ln2")
                        # -- LN2 stats over 4608 merged features (per window)
                        st2 = ps_b.tile([1, 2 * TC], F32, tag="bmm",
                                        name=f"st2_{c}")
                        for dt in range(DT):
                            ysq = st_pool.tile([128, TC], BF16, tag="xsq", bufs=1)
                            nc.vector.tensor_mul(ysq, y[:, dt], y[:, dt])
                            nc.tensor.matmul(st2[:, :TC], ones_col, y[:, dt],
                                             start=(dt == 0), stop=(dt == DT - 1))
                            nc.tensor.matmul(st2[:, TC:], ones_col, ysq,
                                             start=False, stop=(dt == DT - 1))

                        # fold the 4 a-positions: [1, (a w)] -> [1, w]
                        s2a = st_pool.tile([1, WC], F32, tag="s2a")
                        s2b = st_pool.tile([1, WC], F32, tag="s2b")
                        nc.vector.tensor_reduce(
                            s2a, st2[:, :TC].rearrange("p (a w) -> p w a", a=4),
                            axis=mybir.AxisListType.X, op=ALU.add)
                        nc.vector.tensor_reduce(
                            s2b, st2[:, TC:].rearrange("p (a w) -> p w a", a=4),
                            axis=mybir.AxisListType.X, op=ALU.add)
                        u2 = st_pool.tile([1, WC], F32, tag="t3")
                        nc.gpsimd.tensor_scalar_mul(u2, s2a, 1.0 / D4)
                        usq = st_pool.tile([1, WC], F32, tag="usq")
                        nc.gpsimd.tensor_mul(usq, u2, u2)
                        var2 = st_pool.tile([1, WC], F32, tag="var2")
                        nc.gpsimd.tensor_scalar_mul(var2, s2b, 1.0 / D4)
                        nc.gpsimd.tensor_sub(var2, var2, usq)
                        rstd2 = st_pool.tile([1, WC], BF16, tag="rstd2")
                        _rsqrt(nc, st_pool, rstd2, var2, "r2", eng=nc.gpsimd)
                        pmu2 = st_pool.tile([1, WC], BF16, tag="nmu2")
                        with nc.allow_low_precision(reason="ln2 scale bf16"):
                            nc.gpsimd.tensor_mul(pmu2, u2, rstd2)

                        rstd2_b = st_pool.tile([128, WC], BF16, tag="rstd2_b")
                        pmu2_b = st_pool.tile([128, WC], BF16, tag="nmu2_b")
                        nc.gpsimd.partition_broadcast(rstd2_b, rstd2)
                        nc.gpsimd.partition_broadcast(pmu2_b, pmu2)
                        if ilv:
                            emit_bitem()

                        _mark(nc, f"A{c}:h2c")
                        # -- h2 = y*rstd2 + nmu2 (bf16); seed acc w/ residual
                        # chunks with slack before stage B reads h2 ride the
                        # idle Pool engine; gating chunks stay on DVE
                        h2eng = nc.vector if c in (3, CH - 1) else nc.gpsimd
                        for dt in range(DT):
                            tmp2 = st_pool.tile([128, 4, WC], BF16,
                                                tag="normtmp2", bufs=1)
                            h2eng.tensor_mul(
                                tmp2, y[:, dt].rearrange("p (a w) -> p a w", a=4),
                                rstd2_b.unsqueeze(1).to_broadcast([128, 4, WC]))
                            h2eng.tensor_sub(
                                h2[:, dt, :, w0:w0 + WC], tmp2,
                                pmu2_b.unsqueeze(1).to_broadcast([128, 4, WC]))
                            rs = st_pool.tile([128, WC], BF16, tag="ressc",
                                              bufs=1, name=f"rs{c}_{dt}")
                            with nc.allow_low_precision(reason="residual bf16"):
                                nc.vector.tensor_reduce(
                                    rs, y[:, dt].rearrange("p (a w) -> p w a", a=4),
                                    axis=mybir.AxisListType.X, op=ALU.add)
                            nc.gpsimd.tensor_scalar_mul(
                                acc[:, dt, w0:w0 + WC], rs, 0.25)
                        if ilv:
                            emit_bitem()
                    return emit_ln2

                # defer this chunk's LN2+h2c past the next chunk's scores so
                # the DVE queue serves the scores products first
                pend_ln2[0] = make_ln2(c, xc, y, w0, ilv)
                if c == CH - 1:
                    pend_ln2[0]()
                    pend_ln2[0] = None
                if c == 2:
                    load_consts3()
                if c == 3:
                    load_w(0)
                    load_w(1)

        if debug:
            nc.sync.dma_start(dbg["dbg_h2"], h2)
            pass

        # =================== Stage B: merger MLP (bulk) ====================
        with ExitStack() as bctx:
            m2f_pool[0] = bctx.enter_context(tc.tile_pool(name="m2f", bufs=2))

            _mark(nc, "Bbulk")
            bbudget[0] = 10 ** 9
            while pi[0] < len(items):
                emit_bitem()

            if debug:
                nc.sync.dma_start(dbg["dbg_acc"], acc)

    nc.compile()
    return nc


# ---------------------------------------------------------------------------
# Host side
# ---------------------------------------------------------------------------

_CACHED = {}


def make_runner(nc):
    """Build a reusable jitted SPMD executor for the finalized program.

    Mirrors concourse.bass2jax.run_bass_via_pjrt but caches the jitted
    callable so repeated kernel() calls (and benchmarking) don't recompile.
    Returns run(in_maps) -> list[dict] per core.
    """
    import jax
    from jax.sharding import Mesh, PartitionSpec
    from jax.experimental.shard_map import shard_map
    from concourse import mybir as _mybir
    from concourse.bass2jax import (
        install_neuronx_cc_hook, partition_id_tensor, _bass_exec_p)

    install_neuronx_cc_hook()
    partition_name = nc.partition_id_tensor.name if nc.partition_id_tensor else None

    in_names, out_names, out_avals, zero_shapes = [], [], [], []
    for alloc in nc.m.functions[0].allocations:
        if not isinstance(alloc, _mybir.MemoryLocationSet):
            continue
        name = alloc.memorylocations[0].name
        if alloc.kind == "ExternalInput":
            if name != partition_name:
                in_names.append(name)
        elif alloc.kind == "ExternalOutput":
            out_names.append(name)
            shape = tuple(alloc.tensor_shape)
            dtype = _mybir.dt.np(alloc.dtype)
            out_avals.append(jax.core.ShapedArray(shape, dtype))
            zero_shapes.append((shape, dtype))

    n_params = len(in_names)
    n_outs = len(out_avals)
    all_in_names = list(in_names) + list(out_names)
    if partition_name is not None:
        all_in_names.append(partition_name)
    donate = tuple(range(n_params, n_params + n_outs))

    def _body(*args):
        operands = list(args)
        if partition_name is not None:
            operands.append(partition_id_tensor())
        outs = _bass_exec_p.bind(
            *operands,
            out_avals=tuple(out_avals),
            in_names=tuple(all_in_names),
            out_names=tuple(out_names),
            lowering_input_output_aliases=(),
            sim_require_finite=True,
            sim_require_nnan=True,
            nc=nc,
        )
        return tuple(outs)

    devices = jax.devices()[:NCORES]
    mesh = Mesh(np.asarray(devices), ("core",))
    in_specs = (PartitionSpec("core"),) * (n_params + n_outs)
    out_specs = (PartitionSpec("core"),) * n_outs
    sharded = jax.jit(
        shard_map(_body, mesh=mesh, in_specs=in_specs, out_specs=out_specs,
                  check_rep=False),
        donate_argnums=donate, keep_unused=True)

    def make_zeros():
        return [np.zeros((NCORES * s[0], *s[1:]), d) for s, d in zero_shapes]

    def concat_inputs(in_maps):
        return [np.concatenate([np.asarray(in_maps[c][n]) for c in range(NCORES)],
                               axis=0)
                for n in in_names]

    def run(in_maps):
        out_arrs = sharded(*concat_inputs(in_maps), *make_zeros())
        return [
            {n: np.asarray(out_arrs[i]).reshape(NCORES, *out_avals[i].shape)[c]
             for i, n in enumerate(out_names)}
            for c in range(NCORES)
        ]

    run.sharded = sharded
    run.concat_inputs = concat_inputs
    run.make_zeros = make_zeros
    run.out_names = out_names
    run.out_avals = out_avals
    return run


def _prep_weights(ln1_g, ln1_b, w_qkv, b_qkv, w_o, b_o, pre_g, pre_b, w1, b1, w2, b2):
    bf = ml_dtypes.bfloat16
    f32 = np.float32

    ln1_g = np.asarray(ln1_g, f32)
    ln1_b = np.asarray(ln1_b, f32)
    w_qkv = np.asarray(w_qkv, f32)
    w1 = np.asarray(w1, f32)
    w2 = np.asarray(w2, f32)
    w_o = np.asarray(w_o, f32)
    pre_g = np.asarray(pre_g, f32)
    pre_b = np.asarray(pre_b, f32)

    f8 = ml_dtypes.float8_e4m3
    wq = w_qkv * ln1_g[None, :]
    bq = w_qkv @ ln1_b + np.asarray(b_qkv, f32)
    wqkv_t = np.ascontiguousarray(
        (wq.T * S_WQ).reshape(DT, 128, JQ, 128).transpose(1, 2, 0, 3)).astype(f8)
    bqkv_h = np.ascontiguousarray(bq.reshape(JQ, 128).T)

    wo_t = np.ascontiguousarray(
        (w_o.T * S_WO).reshape(DT, 128, D).transpose(1, 0, 2)).astype(f8)
    bo_h = np.ascontiguousarray(np.asarray(b_o, f32).reshape(DT, 128).T)

    w1g = w1 * pre_g[None, :]
    b1e = w1 @ pre_b + np.asarray(b1, f32)
    w1p = np.zeros((J1P, D4), f32)
    w1p[:J1] = w1g
    w1_t = np.ascontiguousarray(
        w1p.T.reshape(KT1, 128, JT1, 128).transpose(2, 1, 0, 3)).astype(bf)
    b1p = np.zeros((J1P,), f32)
    b1p[:J1] = b1e
    b1_h = np.ascontiguousarray(b1p.reshape(JT1, 128).T)

    w2p = np.zeros((J1P, D), f32)
    w2p[:J1] = w2.T
    w2_t = np.ascontiguousarray(
        w2p.reshape(JT1, 128, DT, 128).transpose(2, 1, 0, 3)).astype(bf)
    b2_h = np.ascontiguousarray(np.asarray(b2, f32).reshape(DT, 128).T)

    heads = (np.arange(D) // HD)
    obd = (heads[:, None] == np.arange(NH)[None, :]).astype(bf)      # [D, NH]
    obd_h = np.ascontiguousarray(obd.reshape(DT, 128, NH).transpose(1, 0, 2))
    obdT_h = np.ascontiguousarray(obd.T.reshape(NH, DT, 128))

    return dict(
        wqkv=wqkv_t, bqkv=bqkv_h, wo=wo_t, bo=bo_h,
        ones_bd=obd_h, ones_bdT=obdT_h,
        w1t=w1_t, b1=b1_h, w2t=w2_t, b2=b2_h,
    )


def _shard_x(hidden_states):
    """Full x [1, T, D] -> per-core chunk-major bf16 [CH, 128, DT, 4*WC]."""
    bf = ml_dtypes.bfloat16
    x = np.asarray(hidden_states, np.float32)[0]          # [T, D]
    nh, nw = H // 2, W // 2
    xr = x.reshape(B, nh, 2, nw, 2, D)
    shards = []
    for c in range(NCORES):
        img, half = divmod(c, 2)
        sl = xr[img, half * 16:(half + 1) * 16]           # [16, 2, 32, 2, D]
        # (a=(r,cc), w=(i,j)) ordering
        sl = sl.transpose(1, 3, 0, 2, 4).reshape(TS, D)   # [(r c i j), D]
        xT = np.ascontiguousarray(sl.T).reshape(DT, 128, 4, NW)
        # chunk-major, partition-leading: [CH, 128, DT, 4*WC] so the fused
        # DMA iterates src and dst in the same (p, dt, col) order
        xh = np.ascontiguousarray(
            xT.reshape(DT, 128, 4, CH, WC).transpose(3, 1, 0, 2, 4)
        ).reshape(CH, 128, DT, 4 * WC).astype(bf)
        shards.append(xh)
    return shards


def get_runner():
    if "runner" not in _CACHED:
        nc = build_program()
        _CACHED["runner"] = make_runner(nc)
    return _CACHED["runner"]


def make_in_maps(inputs):
    weights = _prep_weights(
        inputs["ln1_g"], inputs["ln1_b"], inputs["w_qkv"], inputs["b_qkv"],
        inputs["w_o"], inputs["b_o"], inputs["pre_g"], inputs["pre_b"],
        inputs["w1"], inputs["b1"], inputs["w2"], inputs["b2"])
    shards = _shard_x(inputs["hidden_states"])
    return [dict(weights, xT=shards[c]) for c in range(NCORES)]


def kernel(**inputs):
    run = get_runner()
    results = run(make_in_maps(inputs))
    # per-core out is feature-major [DT, 128, NW]; transpose to [NW, D]
    outs = [np.asarray(results[c]["out"]).reshape(D, NW).T
            for c in range(NCORES)]
    out = np.concatenate(outs, axis=0)
    return out[None].astype(np.float32)



# revision 114
# speedup vs baseline: 1.0602x; 1.0602x over previous
"""Trainium2 Bass kernel for MiniCPMV ViT window-attention + 2x2 merger block.

Architecture (per reference):
  x[1,16384,1152] -> LN1 -> 2x2-window reorder -> QKV -> 4-token window attn
  (16 heads x 72) -> out-proj -> un-reorder + residual -> re-reorder ->
  [4096 windows x 4608] -> LN2 -> Linear(4608->17216) -> gelu(tanh) ->
  Linear(17216->1152) -> + mean-pool residual -> [1,4096,1152]

Key observation: the un-reorder after attention and the re-reorder before the
merger cancel, so everything stays in window order end-to-end and the output
is already in window (= merged token) order.

Sharding: pure data parallel over 8 cores; each core takes 2048 tokens
(512 windows, half of one image = 16 window-rows), weights replicated.
Token order within a core is (a, w): a = position-in-window (0..3),
w = window index (0..511), so per-a slices are contiguous.

On-chip layout is feature-major ([d on partitions, tokens on free axis]);
host pre-transposes x (bf16, chunk-major partition-leading so each chunk is
two fused DMAs) and all weights. QKV and the out-projection run in fp8-e4m3
DoubleRow mode (2 k-tiles per matmul; weights pre-scaled x32 host-side, the
scale folded out at PSUM evacuation); W1/W2 stay bf16 for the 2e-2 accuracy
budget. LN stat chains and h2 writes ride the otherwise-idle Pool (gpsimd)
engine; each chunk's LN2 is deferred past the next chunk's scores products
so the DVE queue serves the attention chain first. Stage-B (merger MLP)
weights stream on the Activation HWDGE queue, prefetched one item ahead, and
block 0 is interleaved into late stage-A chunks; the last beta block and the
gamma half-block alternate their W2/output chains so PE covers the final acc
updates. Output leaves the device feature-major and is transposed on host.
"""

import numpy as np
import ml_dtypes

import concourse.bacc as bacc
import concourse.tile as tile
import concourse.bass as bass
from concourse import mybir

F32 = mybir.dt.float32
BF16 = mybir.dt.bfloat16
F8 = mybir.dt.float8e4
I32 = mybir.dt.int32
AF = mybir.ActivationFunctionType
ALU = mybir.AluOpType
DR = mybir.MatmulPerfMode.DoubleRow

# Problem constants (hardcoded per spec)
B, H, W, D, I, NH = 4, 64, 64, 1152, 4304, 16
T = B * H * W          # 16384 tokens
HD = D // NH           # 72 head dim
NCORES = 8
TS = T // NCORES       # 2048 tokens per core
NW = TS // 4           # 512 windows per core
DT = D // 128          # 9 feature tiles
JQ = 3 * D // 128      # 27 qkv output tiles
D4 = 4 * D             # 4608 merged feature dim
KT1 = D4 // 128        # 36 contraction tiles for w1
J1 = 4 * I             # 17216
J1P = 17280            # padded to 135*128
JT1 = J1P // 128       # 135
JBLK = 15              # w1 j-tiles per block
NBLK = JT1 // JBLK     # 9 blocks
CH = 8                 # stage-A chunks
WC = NW // CH          # 64 windows per chunk
TC = 4 * WC            # 256 tokens per chunk
EPS = 1e-6
SM_SCALE = 1.0 / np.sqrt(HD)
S_WQ = 32.0               # fp8 qkv weight pre-scale (folded out at PSUM evac)
S_WO = 32.0               # fp8 out-proj weight pre-scale

PHASE_MARKS = []  # (label, first_instruction_index); for profiling only


def _mark(nc, label):
    PHASE_MARKS.append((label, int(nc.get_next_instruction_name()[2:])))


def _rsqrt(nc, st_pool, out, z, tag, iters=3, eng=None, sdt=None):
    """out = 1/sqrt(z) via Newton from y0=1 (valid for z in ~(0.1, 3)).

    LN variances here are ~1 so a constant init converges in 3 iterations.
    Four scratch tags scheduled so no tag is reused while its tile is still
    an input of a later instruction (safe with bufs=1 pools).
    z: [1, N] f32 SBUF; out: [1, N] bf16.
    """
    ve = eng if eng is not None else nc.vector
    dt_ = sdt if sdt is not None else F32
    n = z.shape[-1]
    seq = [0, 1, 2, 3, 1, 2, 3, 0, 1, 2, 3]  # tag index per scratch alloc
    ti = 0

    def scratch():
        nonlocal ti
        t = st_pool.tile([1, n], dt_, tag=f"{tag}_{seq[ti]}", name=f"{tag}_s{ti}", bufs=1)
        ti += 1
        return t

    with nc.allow_low_precision(reason="rsqrt scratch"):
        # y1 = 1.5 - 0.5 z   (Newton step from y0 = 1)
        y = scratch()
        ve.tensor_scalar(y, z, -0.5, 1.5, ALU.mult, ALU.add)
        for it in range(1, iters):
            t = scratch()
            ve.tensor_mul(t, z, y)
            t2 = scratch()
            ve.tensor_mul(t2, t, y)
            u = scratch()
            ve.tensor_scalar(u, t2, -0.5, 1.5, ALU.mult, ALU.add)
            if it < iters - 1:
                yn = scratch()
                ve.tensor_mul(yn, u, y)
            else:
                yn = out
                ve.tensor_mul(yn, u, y)
            y = yn


def build_program(debug=False):
    """Build the single-core SPMD program (same NEFF on all 8 cores)."""
    from contextlib import ExitStack
    PHASE_MARKS.clear()

    nc = bacc.Bacc("TRN2", target_bir_lowering=False, num_devices=NCORES)

    # ---- DRAM parameters -------------------------------------------------
    def inp(name, shape, dtype):
        return nc.dram_tensor(name, shape, dtype, kind="ExternalInput").ap()

    xT_d = inp("xT", [CH, 128, DT, 4 * WC], BF16)   # chunk-major, partition-leading
    wqkv_d = inp("wqkv", [128, JQ, DT, 128], F8)    # [p][jt][dt][col], x S_WQ
    bqkv_d = inp("bqkv", [128, JQ], F32)
    wo_d = inp("wo", [128, DT, D], F8)              # [p][dtk][j], x S_WO
    bo_d = inp("bo", [128, DT], F32)
    obd_d = inp("ones_bd", [128, DT, NH], BF16)     # block-diag head masks
    obdT_d = inp("ones_bdT", [NH, DT, 128], BF16)
    w1_d = inp("w1t", [JT1, 128, KT1, 128], BF16)   # [jt][p][kt][col]
    b1_d = inp("b1", [128, JT1], F32)
    w2_d = inp("w2t", [DT, 128, JT1, 128], BF16)    # [dt][p][jt][col]
    b2_d = inp("b2", [128, DT], F32)

    # feature-major output [dt][p][w]; host transposes to [NW, D] after gather
    out_d = nc.dram_tensor("out", [DT, 128, NW], F32, kind="ExternalOutput").ap()
    if debug:
        dbg = {
            "dbg_qk": nc.dram_tensor("dbg_qk", [128, 2 * DT, TC], BF16,
                                     kind="ExternalOutput").ap(),
            "dbg_h": nc.dram_tensor("dbg_h", [128, DT, TC], F8,
                                    kind="ExternalOutput").ap(),
            "dbg_xc": nc.dram_tensor("dbg_xc", [128, DT, 4, WC], BF16,
                                     kind="ExternalOutput").ap(),
            "dbg_v": nc.dram_tensor("dbg_v", [128, DT, TC], BF16,
                                    kind="ExternalOutput").ap(),
            "dbg_attn": nc.dram_tensor("dbg_attn", [NH, 4, TS // 4 // CH, 4], BF16,
                                       kind="ExternalOutput").ap(),
            "dbg_y": nc.dram_tensor("dbg_y", [128, DT, 4, TS // 4 // CH], BF16,
                                    kind="ExternalOutput").ap(),
            "dbg_h2": nc.dram_tensor("dbg_h2", [128, DT, 4, NW], BF16,
                                     kind="ExternalOutput").ap(),
            "dbg_res": nc.dram_tensor("dbg_res", [128, DT, NW], BF16,
                                      kind="ExternalOutput").ap(),
            "dbg_acc": nc.dram_tensor("dbg_acc", [128, DT, NW], F32,
                                      kind="ExternalOutput").ap(),
        }

    with tile.TileContext(nc) as tc, ExitStack() as ctx:
        # ---- pools -------------------------------------------------------
        consts = ctx.enter_context(tc.tile_pool(name="consts", bufs=1))
        persist = ctx.enter_context(tc.tile_pool(name="persist", bufs=1))

        # ---- constants (DMAs deferred until after chunk 0's x-load) ------
        wo_sb = consts.tile([128, DT, D], F8)
        bqkv_sb = consts.tile([128, JQ], F32)
        bo_sb = consts.tile([128, DT], F32)
        b1_sb = consts.tile([128, JT1], F32)
        b2_sb = consts.tile([128, DT], F32)
        obd_sb = consts.tile([128, DT, NH], BF16)
        obdT_sb = consts.tile([NH, DT, 128], BF16)

        def load_consts():
            nc.sync.dma_start(obd_sb, obd_d)
            nc.sync.dma_start(bqkv_sb, bqkv_d)

        def load_consts2():
            nc.sync.dma_start(obdT_sb, obdT_d)
            nc.sync.dma_start(wo_sb, wo_d)
            nc.sync.dma_start(bo_sb, bo_d)

        def load_consts3():
            nc.sync.dma_start(b1_sb, b1_d)
            nc.sync.dma_start(b2_sb, b2_d)

        ones_col = consts.tile([128, 1], BF16)
        nc.vector.memset(ones_col, 1.0)

        # attention -> merger handoff, kept in SBUF
        h2 = persist.tile([128, DT, 4, NW], BF16)    # LN2-normalized y

        # ---- stage-B shared pools (used interleaved with A and after) ----
        acc_pool = ctx.enter_context(tc.tile_pool(name="acc", bufs=1))
        w1_pool = ctx.enter_context(tc.tile_pool(name="w1s", bufs=2))
        m2h_pool = ctx.enter_context(tc.tile_pool(name="m2h", bufs=1))
        w2_pool = ctx.enter_context(tc.tile_pool(name="w2s", bufs=3))
        ps_b = ctx.enter_context(tc.tile_pool(name="ps_b", bufs=2, space="PSUM"))
        m2f_pool = [None]

        acc = acc_pool.tile([128, DT, NW], F32)
        HW1 = NW // 2          # window-half size (256)
        NIB = 1                # blocks interleaved into stage A at N=HW1

        def w1_compute(jt, wlo, n, dst, w1s, sfx):
            """One W1 j-tile over window range [wlo, wlo+n) -> dst [128, n]."""
            mm = ps_b.tile([128, n], F32, tag="bmm", name=f"bmm{sfx}")
            for kt in range(KT1):
                a, dt = divmod(kt, DT)
                nc.tensor.matmul(mm, w1s[:, kt], h2[:, dt, a, wlo:wlo + n],
                                 start=(kt == 0), stop=(kt == KT1 - 1))
            nc.scalar.activation(dst, mm, AF.Gelu_apprx_tanh,
                                 bias=b1_sb[:, jt:jt + 1])

        def acc_update(dt, mm_ap, wlo, n, first, last):
            # acc is pre-seeded with the mean-pool residual at h2c time, so
            # every block update is an add (the last one also adds the bias)
            accs = acc[:, dt, wlo:wlo + n]
            if last:
                nc.vector.scalar_tensor_tensor(
                    accs, mm_ap, b2_sb[:, dt:dt + 1], accs, ALU.add, ALU.add)
            else:
                nc.vector.tensor_add(accs, mm_ap, accs)

        def fin_dt(dt, mts):
            # output is feature-major: DMA the finished window-half directly
            lo, hi = mts[0] * 128, (mts[-1] + 1) * 128
            nc.sync.dma_start(out_d[dt, :, lo:hi], acc[:, dt, lo:hi])

        def w2_compute(blk, dt, wlo, n, m2t, flags, w2s, fin_mts, sfx):
            """W2 for one (block, dt) over [wlo, wlo+n); flags: [(lo,sz,first,last)].

            fin_mts: if set, emit that dt's output DMA right after its
            last acc update (folds the output tail into the last block).
            """
            mm = ps_b.tile([128, n], F32, tag="bmm", name=f"w2mm{sfx}_{dt}")
            for j in range(JBLK):
                nc.tensor.matmul(mm, w2s[:, j], m2t[:, j],
                                 start=(j == 0), stop=(j == JBLK - 1))
            for lo, sz, first, last in flags:
                acc_update(dt, mm[:, lo - wlo:lo - wlo + sz], lo, sz,
                           first, last)
            if fin_mts is not None:
                fin_dt(dt, fin_mts)

        # ---- unified stage-B worklist: interleave / beta / gamma ----------
        # item: ("w1"/"w1g", blk, j, lo, n) or ("w2", blk, dt, lo, n, flags, fin)
        items = []
        for blk in range(NIB):          # interleave: block 0, window half 1
            for j in range(JBLK):
                items.append(("w1", blk, j, 0, HW1))
            for dt in range(DT):
                items.append(("w2", blk, dt, 0, HW1,
                              [(0, HW1, blk == 0, False)], None))
        N_ILV = len(items)              # only these may be emitted inside A
        for blk in range(NIB, NBLK - 1):  # beta: full-N blocks
            for j in range(JBLK):
                # first beta items compute window half 1 first so they don't
                # stall on the final h2 chunk right at the A->B boundary
                kind = "w1x" if blk == NIB and j < 2 else "w1"
                items.append((kind, blk, j, 0, NW))
            for dt in range(DT):
                items.append(("w2", blk, dt, 0, NW,
                              [(0, HW1, False, False),
                               (HW1, HW1, blk == NIB, False)],
                              None))
        # tail: last beta block (B{NBLK-1}) + gamma (block 0 half 2) with
        # their w2/fin chains interleaved so PE covers each other's acc DVE
        blk = NBLK - 1
        for j in range(JBLK):
            items.append(("w1", blk, j, 0, NW))
        for j in range(JBLK):
            items.append(("w1g", 0, j, HW1, HW1))
        for dt in range(DT):
            items.append(("w2", blk, dt, 0, NW,
                          [(0, HW1, False, True), (HW1, HW1, False, False)],
                          [0, 1]))
            items.append(("w2", 0, dt, HW1, HW1,
                          [(HW1, HW1, False, True)],
                          [2, 3]))

        m2_of = {}
        wpre = {}
        bbudget = [0]
        pi = [0]

        def load_w(i):
            """Issue the weight DMA for item i (idempotent prefetch)."""
            if i >= len(items) or i in wpre:
                return
            it = items[i]
            # stage-B weights ride the Activation HWDGE queue so they never
            # delay the latency-sensitive x/wq stream on the SP queue
            if it[0] != "w2":
                jt = it[1] * JBLK + it[2]
                t = w1_pool.tile([128, KT1, 128], BF16, tag="w1s",
                                 name=f"w1s_{i}")
                nc.scalar.dma_start(t, w1_d[jt])
            else:
                blk, dt = it[1], it[2]
                t = w2_pool.tile([128, JBLK, 128], BF16, tag="w2s",
                                 name=f"w2s_{i}")
                nc.scalar.dma_start(t, w2_d[dt, :, blk * JBLK:(blk + 1) * JBLK])
            wpre[i] = t

        def emit_bitem():
            if pi[0] >= len(items) or bbudget[0] <= 0:
                return
            bbudget[0] -= 1
            i = pi[0]
            pi[0] += 1
            it = items[i]
            load_w(i)
            load_w(i + 1)
            if i + 2 < len(items) and items[i + 2][0] == "w2" and \
                    items[i + 1][0] == "w2":
                load_w(i + 2)   # w2 pool has 3 bufs: run 2 ahead
            if it[0] == "w2" and it[2] in (0, 3, 6):
                # w1 bufs are both free during a block's w2 run: top up the
                # next block's first w1 weights early. Cap outstanding w1
                # prefetches at 2 (pool bufs) or the blocked DMA would sit at
                # the queue head and stall the w2 weight stream behind it.
                n_w1_pre = sum(1 for k2 in wpre if items[k2][0] != "w2")
                if n_w1_pre < 2:
                    for k in range(i + 2, min(i + 16, len(items))):
                        if items[k][0] != "w2" and k not in wpre:
                            load_w(k)
                            break
            w = wpre.pop(i)
            if it[0] != "w2":
                _, blk, j, lo, n = it
                if j == 0:
                    if n == HW1:
                        m2_of[blk] = m2h_pool.tile([128, JBLK, HW1], BF16,
                                                   tag="m2h", name=f"m2h{i}")
                    else:
                        m2_of[blk] = m2f_pool[0].tile([128, JBLK, NW], BF16,
                                                      tag="m2f", name=f"m2f{i}")
                if it[0] == "w1x":
                    for half in range(2):
                        w1_compute(blk * JBLK + j, half * HW1, HW1,
                                   m2_of[blk][:, j, half * HW1:(half + 1) * HW1],
                                   w, f"b{i}_{half}")
                else:
                    w1_compute(blk * JBLK + j, lo, n, m2_of[blk][:, j], w,
                               f"b{i}")
            else:
                _, blk, dt, lo, n, flags, fin = it
                w2_compute(blk, dt, lo, n, m2_of[blk], flags, w, fin, f"b{i}")
                if dt == DT - 1:
                    m2_of.pop(blk)

        # =================== Stage A: LN1 + attention =====================
        with ExitStack() as actx:
            xc_pool = actx.enter_context(tc.tile_pool(name="xc", bufs=3))
            wq_pool = actx.enter_context(tc.tile_pool(name="wq", bufs=4))
            h_pool = actx.enter_context(tc.tile_pool(name="h", bufs=2))
            ob_pool = actx.enter_context(tc.tile_pool(name="ob", bufs=1))
            qk_pool = actx.enter_context(tc.tile_pool(name="qk", bufs=1))
            v_pool = actx.enter_context(tc.tile_pool(name="v", bufs=2))
            p_pool = actx.enter_context(tc.tile_pool(name="p", bufs=2))
            sm_pool = actx.enter_context(tc.tile_pool(name="sm", bufs=2))
            av_pool = actx.enter_context(tc.tile_pool(name="av", bufs=2))
            st_pool = actx.enter_context(tc.tile_pool(name="st", bufs=2))
            st1_pool = actx.enter_context(tc.tile_pool(name="st1b", bufs=1))
            ps_mm = actx.enter_context(tc.tile_pool(name="ps_mm", bufs=3, space="PSUM"))
            ps_work = actx.enter_context(tc.tile_pool(name="ps_work", bufs=3, space="PSUM"))

            def ln_stage(c):
                """Emit x-load + LN1 + normalize for chunk c; returns (xc, h)."""
                w0 = c * WC
                _mark(nc, f"A{c}:xload")
                # -- load x chunk: [128, dt, a, WC] bf16, two fused DMAs
                xc = xc_pool.tile([128, DT, 4, WC], BF16, tag="xc", name=f"xc{c}")
                xf = xc.rearrange("p d a w -> p d (a w)")
                xs = xT_d[c]
                if c == 0:
                    # finer granularity at cold start: stats can begin as soon
                    # as the first dt tiles land
                    for lo, hi in ((0, 3), (3, 6), (6, 9)):
                        nc.sync.dma_start(xf[:, lo:hi], xs[:, lo:hi])
                else:
                    nc.sync.dma_start(xf[:, :5], xs[:, :5])
                    nc.sync.dma_start(xf[:, 5:], xs[:, 5:])
                if debug and c == 0:
                    nc.sync.dma_start(dbg["dbg_xc"], xc)

                _mark(nc, f"A{c}:ln1")
                # -- LN1 stats: col sums of x and x^2 via ones-vector matmul
                st1 = ps_work.tile([1, 2 * TC], F32, tag="work", name=f"st1_{c}")
                for dt in range(DT):
                    xflat = xc[:, dt].rearrange("p a w -> p (a w)")
                    xsq = st_pool.tile([128, TC], BF16, tag="xsq", name=f"xsq{c}_{dt}", bufs=1)
                    nc.vector.tensor_mul(xsq, xflat, xflat)
                    # NOTE: start=True clears has_written for the WHOLE psum
                    # bank, so only the bank's first matmul may set it; other
                    # groups overwrite-on-first-touch via the cleared bits.
                    nc.tensor.matmul(st1[:, :TC], ones_col, xflat,
                                     start=(dt == 0), stop=(dt == DT - 1))
                    nc.tensor.matmul(st1[:, TC:], ones_col, xsq,
                                     start=False, stop=(dt == DT - 1))
                stx, stq = st1[:, :TC], st1[:, TC:]

                # var = stq/D - (stx/D)^2 ; rstd = 1/sqrt(var) via DVE Newton
                # (DVE ops may read at most one PSUM input -> evac via ts_mul)
                m1t = st1_pool.tile([1, TC], F32, tag="m1t", name=f"m1t_{c}")
                nc.vector.tensor_scalar_mul(m1t, stx, 1.0 / D)
                var = st1_pool.tile([1, TC], F32, tag="var", name=f"var_{c}")
                nc.vector.tensor_scalar_mul(var, stq, 1.0 / D)
                # (Pool has no scalar_tensor_tensor: use mul/sub pairs and a
                # positive pmu = mean*rstd, subtracted in the normalize)
                t1 = st1_pool.tile([1, TC], F32, tag="r1_0", name=f"t1_{c}")
                nc.gpsimd.tensor_mul(t1, m1t, m1t)
                nc.gpsimd.tensor_sub(var, var, t1)
                rstd = st_pool.tile([1, TC], BF16, tag="rstd", name=f"rstd_{c}")
                _rsqrt(nc, st1_pool, rstd, var, "r1", iters=2, eng=nc.gpsimd,
                       sdt=BF16)
                pmu = st_pool.tile([1, TC], BF16, tag="nmu", name=f"nmu_{c}")
                with nc.allow_low_precision(reason="ln scale bf16"):
                    nc.gpsimd.tensor_mul(pmu, m1t, rstd)

                rstd_b = st_pool.tile([128, TC], BF16, tag="rstd_b", name=f"rstdb_{c}")
                pmu_b = st_pool.tile([128, TC], BF16, tag="nmu_b", name=f"nmub_{c}")
                nc.gpsimd.partition_broadcast(rstd_b, rstd)
                nc.gpsimd.partition_broadcast(pmu_b, pmu)

                _mark(nc, f"A{c}:norm")
                # -- normalize -> h fp8 [128, dt, TC] (qkv runs in fp8 DoubleRow)
                h = h_pool.tile([128, DT, TC], F8, tag="h", name=f"h{c}")
                for dt in range(DT):
                    tmp = st_pool.tile([128, TC], BF16, tag="normtmp", name=f"nt{c}_{dt}", bufs=1)
                    nc.vector.tensor_mul(tmp, xc[:, dt].rearrange("p a w -> p (a w)"),
                                         rstd_b)
                    with nc.allow_low_precision(reason="qkv input fp8"):
                        nc.vector.tensor_sub(h[:, dt], tmp, pmu_b)
                return xc, h

            qkv_tiles = {}
            wq_pend = {}

            def qkv_stage(c, h, jplo=0, jphi=JQ):
                """QKV matmuls for chunk c (pair range [jplo, jphi))."""
                _mark(nc, f"A{c}:qkv")
                if c not in qkv_tiles:
                    qk = qk_pool.tile([128, 2 * DT, TC], BF16, tag="qk",
                                      name=f"qk{c}")
                    vt = v_pool.tile([128, DT, TC], BF16, tag="v", name=f"v{c}")
                    qkv_tiles[c] = (qk, vt)
                    wq_pend[c] = {}
                qk, vt = qkv_tiles[c]
                def wq_load(jg):
                    """One fused DMA for a group of 4 consecutive j-tiles."""
                    ng = min(4, JQ - jg)
                    wqg = wq_pool.tile([128, 4, DT, 128], F8, tag="wq",
                                       name=f"wqg{c}_{jg}")
                    nc.sync.dma_start(
                        wqg[:, :ng].rearrange("p j d k -> p j (d k)"),
                        wqkv_d[:, jg:jg + ng].rearrange("p j d k -> p j (d k)"))
                    wq_pend[c][jg] = wqg

                for jp in range(jplo, jphi, 2):
                    nj = min(2, JQ - jp)
                    jg = (jp // 4) * 4
                    if jg not in wq_pend[c]:
                        wq_load(jg)
                    if jp == jg and jg + 4 < JQ and jg + 4 not in wq_pend[c]:
                        wq_load(jg + 4)
                    wqg = wq_pend[c][jg]
                    wqs = [wqg[:, jp - jg + j] for j in range(nj)]
                    mm = ps_mm.tile([128, 2, TC], F32, tag="mm0",
                                    name=f"qmm{c}_{jp}")
                    for dtp in range(0, DT, 2):
                        dr = dtp + 2 <= DT
                        for j in range(nj):
                            if dr:
                                nc.tensor.matmul(mm[:, j], wqs[j][:, dtp:dtp + 2],
                                                 h[:, dtp:dtp + 2],
                                                 start=(dtp == 0 and j == 0),
                                                 stop=(dtp + 2 >= DT),
                                                 perf_mode=DR)
                            else:
                                nc.tensor.matmul(mm[:, j], wqs[j][:, dtp],
                                                 h[:, dtp],
                                                 start=(dtp == 0 and j == 0),
                                                 stop=True)
                    for j in range(nj):
                        jt = jp + j
                        dstt = qk[:, jt] if jt < 2 * DT else vt[:, jt - 2 * DT]
                        nc.scalar.activation(dstt, mm[:, j], AF.Identity,
                                             bias=bqkv_sb[:, jt:jt + 1],
                                             scale=1.0 / S_WQ)
                if debug and c == 0 and jphi >= JQ:
                    nc.sync.dma_start(dbg["dbg_qk"], qk)
                    nc.sync.dma_start(dbg["dbg_v"], vt)
                    nc.sync.dma_start(dbg["dbg_h"], h)
                return qk, vt

            # 2-deep software pipeline: LN two chunks ahead, QKV one ahead
            lns = {0: ln_stage(0)}
            load_consts()
            lns[1] = ln_stage(1)
            qkvs = {0: qkv_stage(0, lns[0][1])}
            load_consts2()
            pend_ln2 = [None]
            for c in range(CH):
                w0 = c * WC
                xc, h = lns.pop(c)
                qk, vt = qkvs.pop(c)
                ilv = c >= 4    # h2 half 1 complete -> can interleave B work
                if ilv:
                    rem = max(0, N_ILV - pi[0] - 1)
                    bbudget[0] = -(-rem // (CH - c))  # spread evenly

                _mark(nc, f"A{c}:scores")
                # -- scores: p = q (x) k -> block-diag head reduce -> scs psum
                scs = [ps_work.tile([16, 2, 4, WC], F32, tag="work",
                                    name=f"scs{c}_{i}") for i in range(2)]
                for dt in range(DT):
                    q3 = qk[:, dt].rearrange("p (a w) -> p a w", a=4)
                    k3 = qk[:, DT + dt].rearrange("p (a w) -> p a w", a=4)
                    p_t = p_pool.tile([128, 4, 4, WC], BF16)
                    nc.vector.tensor_mul(
                        p_t,
                        q3.unsqueeze(2).to_broadcast([128, 4, 4, WC]),
                        k3.unsqueeze(1).to_broadcast([128, 4, 4, WC]),
                    )
                    for qi in range(4):
                        nc.tensor.matmul(scs[qi // 2][:, qi % 2], obd_sb[:, dt],
                                         p_t[:, qi].rearrange("p a w -> p (a w)"),
                                         start=(dt == 0 and qi % 2 == 0),
                                         stop=(dt == DT - 1))
                # a few qkv pairs ahead of the deferred LN2 so PE is not
                # blocked in-order on the ln2 stats' ysq dependency
                if c + 1 < CH:
                    qkvs[c + 1] = qkv_stage(c + 1, lns[c + 1][1], 0, 4)
                if pend_ln2[0] is not None:
                    pend_ln2[0]()       # previous chunk's deferred LN2+h2c
                    pend_ln2[0] = None
                if ilv:
                    emit_bitem()

                _mark(nc, f"A{c}:softmax")
                # -- softmax over ki: exp (fused scale), sum, reciprocal, scale
                # layout [h,qi,ki,w]: transpose-free exp evac + packed-w 2x mul
                esb = sm_pool.tile([16, 4, 4, WC], BF16, tag="esb")
                for half in range(2):
                    nc.scalar.activation(
                        esb[:, 2 * half:2 * half + 2], scs[half],
                        AF.Exp, scale=float(SM_SCALE),
                    )
                den = sm_pool.tile([16, 4, WC], BF16, tag="den", bufs=1)
                with nc.allow_low_precision(reason="4-elem softmax sum"):
                    nc.vector.tensor_reduce(den, esb.transpose([0, 1, 3, 2]),
                                            axis=mybir.AxisListType.X,
                                            op=ALU.add)
                rden = sm_pool.tile([16, 4, WC], BF16, tag="rden", bufs=1)
                with nc.allow_low_precision(reason="softmax recip bf16"):
                    nc.vector.reciprocal(rden, den)
                attn = sm_pool.tile([16, 4, 4, WC], BF16, tag="attn")
                nc.vector.tensor_mul(
                    attn, esb,
                    rden.unsqueeze(2).to_broadcast([16, 4, 4, WC]),
                )
                if debug and c == 0:
                    nc.sync.dma_start(dbg["dbg_attn"], attn)
                if c + 1 < CH:
                    qkvs[c + 1] = qkv_stage(c + 1, lns[c + 1][1], 4, 16)
                if ilv:
                    emit_bitem()

                _mark(nc, f"A{c}:av")
                # -- AV: expand attn to feature rows (PE), evac, mul v, 2-add
                o_bf = ob_pool.tile([128, DT, 4, WC], F8, tag="ob", name=f"ob{c}")
                for dt in range(DT):
                    v3 = vt[:, dt].rearrange("p (a w) -> p a w", a=4)
                    exb = av_pool.tile([128, 4, 4, WC], BF16, tag="exb")
                    for half in range(2):
                        ex = ps_work.tile([128, 2, 4, WC], F32, tag="work",
                                          name=f"ex{c}_{dt}_{half}")
                        for qj in range(2):
                            nc.tensor.matmul(
                                ex[:, qj], obdT_sb[:, dt],
                                attn[:, 2 * half + qj],
                                start=True, stop=True,
                            )
                        nc.scalar.activation(exb[:, 2 * half:2 * half + 2], ex,
                                             AF.Identity)
                    prod = av_pool.tile([128, 4, 4, WC], BF16, tag="prod", bufs=1)
                    nc.vector.tensor_mul(
                        prod, exb,
                        v3.unsqueeze(1).to_broadcast([128, 4, 4, WC]),
                    )
                    t2 = av_pool.tile([128, 4, 2, WC], BF16, tag="t2", bufs=1)
                    nc.vector.tensor_add(t2, prod[:, :, 0:2], prod[:, :, 2:4])
                    with nc.allow_low_precision(reason="o fp8 for fp8 outproj"):
                        nc.vector.tensor_add(o_bf[:, dt], t2[:, :, 0],
                                             t2[:, :, 1])
                    if ilv and dt in (3, 7):
                        emit_bitem()

                # independent PE work before outproj (which waits on av DVE)
                if c + 1 < CH:
                    qkvs[c + 1] = qkv_stage(c + 1, lns[c + 1][1], 16, JQ)
                if c + 2 < CH:
                    lns[c + 2] = ln_stage(c + 2)
                if ilv:
                    emit_bitem()

                _mark(nc, f"A{c}:outproj")
                # -- out-projection + bias + residual -> y (in-place into xc)
                # dtk-outer within dto-pairs: PE starts on o_bf[0] early.
                y = xc.rearrange("p d a w -> p d (a w)")
                o_f = o_bf.rearrange("p d a w -> p d (a w)")
                for g in range(0, DT, 2):
                    nd = min(2, DT - g)
                    opt = ps_work.tile([128, 2, TC], F32, tag="work",
                                       name=f"op{c}_{g}")
                    mms = [opt[:, i] for i in range(nd)]
                    for dtp in range(0, DT, 2):
                        dr = dtp + 2 <= DT
                        for i in range(nd):
                            cols = slice((g + i) * 128, (g + i + 1) * 128)
                            if dr:
                                nc.tensor.matmul(
                                    mms[i], wo_sb[:, dtp:dtp + 2, cols],
                                    o_f[:, dtp:dtp + 2],
                                    start=(dtp == 0 and i == 0),
                                    stop=(dtp + 2 >= DT), perf_mode=DR)
                            else:
                                nc.tensor.matmul(
                                    mms[i], wo_sb[:, dtp, cols], o_f[:, dtp],
                                    start=(dtp == 0 and i == 0), stop=True)
                    for i in range(nd):
                        with nc.allow_low_precision(reason="residual stream bf16"):
                            ot = st_pool.tile([128, TC], BF16, tag="opt",
                                              bufs=1, name=f"ot{c}_{g}_{i}")
                            nc.vector.tensor_scalar(
                                ot, mms[i], 1.0 / S_WO,
                                bo_sb[:, g + i:g + i + 1], ALU.mult, ALU.add)
                            nc.vector.tensor_add(y[:, g + i], y[:, g + i], ot)
                if debug and c == 0:
                    nc.sync.dma_start(dbg["dbg_y"], xc)
                if ilv:
                    emit_bitem()

                def make_ln2(c, xc, y, w0, ilv):
                    def emit_ln2():
                        _mark(nc, f"A{c}:ln2")
                        # -- LN2 stats over 4608 merged features (per window)
                        st2 = ps_b.tile([1, 2 * TC], F32, tag="bmm",
                                        name=f"st2_{c}")
                        for dt in range(DT):
                            ysq = st_pool.tile([128, TC], BF16, tag="xsq", bufs=1)
                            nc.vector.tensor_mul(ysq, y[:, dt], y[:, dt])
                            nc.tensor.matmul(st2[:, :TC], ones_col, y[:, dt],
                                             start=(dt == 0), stop=(dt == DT - 1))
                            nc.tensor.matmul(st2[:, TC:], ones_col, ysq,
                                             start=False, stop=(dt == DT - 1))

                        # fold the 4 a-positions: [1, (a w)] -> [1, w]
                        s2a = st_pool.tile([1, WC], F32, tag="s2a")
                        s2b = st_pool.tile([1, WC], F32, tag="s2b")
                        nc.vector.tensor_reduce(
                            s2a, st2[:, :TC].rearrange("p (a w) -> p w a", a=4),
                            axis=mybir.AxisListType.X, op=ALU.add)
                        nc.vector.tensor_reduce(
                            s2b, st2[:, TC:].rearrange("p (a w) -> p w a", a=4),
                            axis=mybir.AxisListType.X, op=ALU.add)
                        u2 = st_pool.tile([1, WC], F32, tag="t3")
                        nc.gpsimd.tensor_scalar_mul(u2, s2a, 1.0 / D4)
                        usq = st_pool.tile([1, WC], F32, tag="usq")
                        nc.gpsimd.tensor_mul(usq, u2, u2)
                        var2 = st_pool.tile([1, WC], F32, tag="var2")
                        nc.gpsimd.tensor_scalar_mul(var2, s2b, 1.0 / D4)
                        nc.gpsimd.tensor_sub(var2, var2, usq)
                        rstd2 = st_pool.tile([1, WC], BF16, tag="rstd2")
                        _rsqrt(nc, st_pool, rstd2, var2, "r2", eng=nc.gpsimd)
                        pmu2 = st_pool.tile([1, WC], BF16, tag="nmu2")
                        with nc.allow_low_precision(reason="ln2 scale bf16"):
                            nc.gpsimd.tensor_mul(pmu2, u2, rstd2)

                        rstd2_b = st_pool.tile([128, WC], BF16, tag="rstd2_b")
                        pmu2_b = st_pool.tile([128, WC], BF16, tag="nmu2_b")
                        nc.gpsimd.partition_broadcast(rstd2_b, rstd2)
                        nc.gpsimd.partition_broadcast(pmu2_b, pmu2)
                        if ilv:
                            emit_bitem()

                        _mark(nc, f"A{c}:h2c")
                        # -- h2 = y*rstd2 + nmu2 (bf16); seed acc w/ residual
                        # chunks with slack before stage B reads h2 ride the
                        # idle Pool engine; gating chunks stay on DVE
                        h2eng = nc.vector if c in (3, CH - 1) else nc.gpsimd
                        for dt in range(DT):
                            tmp2 = st_pool.tile([128, 4, WC], BF16,
                                                tag="normtmp2", bufs=1)
                            h2eng.tensor_mul(
                                tmp2, y[:, dt].rearrange("p (a w) -> p a w", a=4),
                                rstd2_b.unsqueeze(1).to_broadcast([128, 4, WC]))
                            h2eng.tensor_sub(
                                h2[:, dt, :, w0:w0 + WC], tmp2,
                                pmu2_b.unsqueeze(1).to_broadcast([128, 4, WC]))
                            rs = st_pool.tile([128, WC], BF16, tag="ressc",
                                              bufs=1, name=f"rs{c}_{dt}")
                            with nc.allow_low_precision(reason="residual bf16"):
                                nc.vector.tensor_reduce(
                                    rs, y[:, dt].rearrange("p (a w) -> p w a", a=4),
                                    axis=mybir.AxisListType.X, op=ALU.add)
                            nc.gpsimd.tensor_scalar_mul(
                                acc[:, dt, w0:w0 + WC], rs, 0.25)
                        if ilv:
                            emit_bitem()
                    return emit_ln2

                # defer this chunk's LN2+h2c past the next chunk's scores so
                # the DVE queue serves the scores products first
                pend_ln2[0] = make_ln2(c, xc, y, w0, ilv)
                if c == CH - 1:
                    pend_ln2[0]()
                    pend_ln2[0] = None
                if c == 2:
                    load_consts3()
                if c == 3:
                    load_w(0)
                    load_w(1)

        if debug:
            nc.sync.dma_start(dbg["dbg_h2"], h2)
            pass

        # =================== Stage B: merger MLP (bulk) ====================
        with ExitStack() as bctx:
            m2f_pool[0] = bctx.enter_context(tc.tile_pool(name="m2f", bufs=2))

            _mark(nc, "Bbulk")
            bbudget[0] = 10 ** 9
            while pi[0] < len(items):
                emit_bitem()

            if debug:
                nc.sync.dma_start(dbg["dbg_acc"], acc)

    nc.compile()
    return nc


# ---------------------------------------------------------------------------
# Host side
# ---------------------------------------------------------------------------

_CACHED = {}


def make_runner(nc):
    """Build a reusable jitted SPMD executor for the finalized program.

    Mirrors concourse.bass2jax.run_bass_via_pjrt but caches the jitted
    callable so repeated kernel() calls (and benchmarking) don't recompile.
    Returns run(in_maps) -> list[dict] per core.
    """
    import jax
    from jax.sharding import Mesh, PartitionSpec
    from jax.experimental.shard_map import shard_map
    from concourse import mybir as _mybir
    from concourse.bass2jax import (
        install_neuronx_cc_hook, partition_id_tensor, _bass_exec_p)

    install_neuronx_cc_hook()
    partition_name = nc.partition_id_tensor.name if nc.partition_id_tensor else None

    in_names, out_names, out_avals, zero_shapes = [], [], [], []
    for alloc in nc.m.functions[0].allocations:
        if not isinstance(alloc, _mybir.MemoryLocationSet):
            continue
        name = alloc.memorylocations[0].name
        if alloc.kind == "ExternalInput":
            if name != partition_name:
                in_names.append(name)
        elif alloc.kind == "ExternalOutput":
            out_names.append(name)
            shape = tuple(alloc.tensor_shape)
            dtype = _mybir.dt.np(alloc.dtype)
            out_avals.append(jax.core.ShapedArray(shape, dtype))
            zero_shapes.append((shape, dtype))

    n_params = len(in_names)
    n_outs = len(out_avals)
    all_in_names = list(in_names) + list(out_names)
    if partition_name is not None:
        all_in_names.append(partition_name)
    donate = tuple(range(n_params, n_params + n_outs))

    def _body(*args):
        operands = list(args)
        if partition_name is not None:
            operands.append(partition_id_tensor())
        outs = _bass_exec_p.bind(
            *operands,
            out_avals=tuple(out_avals),
            in_names=tuple(all_in_names),
            out_names=tuple(out_names),
            lowering_input_output_aliases=(),
            sim_require_finite=True,
            sim_require_nnan=True,
            nc=nc,
        )
        return tuple(outs)

    devices = jax.devices()[:NCORES]
    mesh = Mesh(np.asarray(devices), ("core",))
    in_specs = (PartitionSpec("core"),) * (n_params + n_outs)
    out_specs = (PartitionSpec("core"),) * n_outs
    sharded = jax.jit(
        shard_map(_body, mesh=mesh, in_specs=in_specs, out_specs=out_specs,
                  check_rep=False),
        donate_argnums=donate, keep_unused=True)

    def make_zeros():
        return [np.zeros((NCORES * s[0], *s[1:]), d) for s, d in zero_shapes]

    def concat_inputs(in_maps):
        return [np.concatenate([np.asarray(in_maps[c][n]) for c in range(NCORES)],
                               axis=0)
                for n in in_names]

    def run(in_maps):
        out_arrs = sharded(*concat_inputs(in_maps), *make_zeros())
        return [
            {n: np.asarray(out_arrs[i]).reshape(NCORES, *out_avals[i].shape)[c]
             for i, n in enumerate(out_names)}
            for c in range(NCORES)
        ]

    run.sharded = sharded
    run.concat_inputs = concat_inputs
    run.make_zeros = make_zeros
    run.out_names = out_names
    run.out_avals = out_avals
    return run


def _prep_weights(ln1_g, ln1_b, w_qkv, b_qkv, w_o, b_o, pre_g, pre_b, w1, b1, w2, b2):
    bf = ml_dtypes.bfloat16
    f32 = np.float32

    ln1_g = np.asarray(ln1_g, f32)
    ln1_b = np.asarray(ln1_b, f32)
    w_qkv = np.asarray(w_qkv, f32)
    w1 = np.asarray(w1, f32)
    w2 = np.asarray(w2, f32)
    w_o = np.asarray(w_o, f32)
    pre_g = np.asarray(pre_g, f32)
    pre_b = np.asarray(pre_b, f32)

    f8 = ml_dtypes.float8_e4m3
    wq = w_qkv * ln1_g[None, :]
    bq = w_qkv @ ln1_b + np.asarray(b_qkv, f32)
    wqkv_t = np.ascontiguousarray(
        (wq.T * S_WQ).reshape(DT, 128, JQ, 128).transpose(1, 2, 0, 3)).astype(f8)
    bqkv_h = np.ascontiguousarray(bq.reshape(JQ, 128).T)

    wo_t = np.ascontiguousarray(
        (w_o.T * S_WO).reshape(DT, 128, D).transpose(1, 0, 2)).astype(f8)
    bo_h = np.ascontiguousarray(np.asarray(b_o, f32).reshape(DT, 128).T)

    w1g = w1 * pre_g[None, :]
    b1e = w1 @ pre_b + np.asarray(b1, f32)
    w1p = np.zeros((J1P, D4), f32)
    w1p[:J1] = w1g
    w1_t = np.ascontiguousarray(
        w1p.T.reshape(KT1, 128, JT1, 128).transpose(2, 1, 0, 3)).astype(bf)
    b1p = np.zeros((J1P,), f32)
    b1p[:J1] = b1e
    b1_h = np.ascontiguousarray(b1p.reshape(JT1, 128).T)

    w2p = np.zeros((J1P, D), f32)
    w2p[:J1] = w2.T
    w2_t = np.ascontiguousarray(
        w2p.reshape(JT1, 128, DT, 128).transpose(2, 1, 0, 3)).astype(bf)
    b2_h = np.ascontiguousarray(np.asarray(b2, f32).reshape(DT, 128).T)

    heads = (np.arange(D) // HD)
    obd = (heads[:, None] == np.arange(NH)[None, :]).astype(bf)      # [D, NH]
    obd_h = np.ascontiguousarray(obd.reshape(DT, 128, NH).transpose(1, 0, 2))
    obdT_h = np.ascontiguousarray(obd.T.reshape(NH, DT, 128))

    return dict(
        wqkv=wqkv_t, bqkv=bqkv_h, wo=wo_t, bo=bo_h,
        ones_bd=obd_h, ones_bdT=obdT_h,
        w1t=w1_t, b1=b1_h, w2t=w2_t, b2=b2_h,
    )


def _shard_x(hidden_states):
    """Full x [1, T, D] -> per-core chunk-major bf16 [CH, 128, DT, 4*WC]."""
    bf = ml_dtypes.bfloat16
    x = np.asarray(hidden_states, np.float32)[0]          # [T, D]
    nh, nw = H // 2, W // 2
    xr = x.reshape(B, nh, 2, nw, 2, D)
    shards = []
    for c in range(NCORES):
        img, half = divmod(c, 2)
        sl = xr[img, half * 16:(half + 1) * 16]           # [16, 2, 32, 2, D]
        # (a=(r,cc), w=(i,j)) ordering
        sl = sl.transpose(1, 3, 0, 2, 4).reshape(TS, D)   # [(r c i j), D]
        xT = np.ascontiguousarray(sl.T).reshape(DT, 128, 4, NW)
        # chunk-major, partition-leading: [CH, 128, DT, 4*WC] so the fused
        # DMA iterates src and dst in the same (p, dt, col) order
        xh = np.ascontiguousarray(
            xT.reshape(DT, 128, 4, CH, WC).transpose(3, 1, 0, 2, 4)
        ).reshape(CH, 128, DT, 4 * WC).astype(bf)
        shards.append(xh)
    return shards


def get_runner():
    if "runner" not in _CACHED:
        nc = build_program()
        _CACHED["runner"] = make_runner(nc)
    return _CACHED["runner"]


def make_in_maps(inputs):
    weights = _prep_weights(
        inputs["ln1_g"], inputs["ln1_b"], inputs["w_qkv"], inputs["b_qkv"],
        inputs["w_o"], inputs["b_o"], inputs["pre_g"], inputs["pre_b"],
        inputs["w1"], inputs["b1"], inputs["w2"], inputs["b2"])
    shards = _shard_x(inputs["hidden_states"])
    return [dict(weights, xT=shards[c]) for c in range(NCORES)]


def kernel(**inputs):
    run = get_runner()
    results = run(make_in_maps(inputs))
    # per-core out is feature-major [DT, 128, NW]; transpose to [NW, D]
    outs = [np.asarray(results[c]["out"]).reshape(D, NW).T
            for c in range(NCORES)]
    out = np.concatenate(outs, axis=0)
    return out[None].astype(np.float32)

